# revision 19
# baseline (speedup 1.0000x reference)
"""Additive (Bahdanau) attention kernel for Trainium2, 8 NeuronCores.

Problem shapes (hardcoded): B=8, TQ=128, TV=256, D=512, U=256.
Sharding: data-parallel over batch B -> one batch element per core.

Per-core algorithm (ACT-bound design; rel tolerance is 2e-2 so the hot
loop runs in bf16 and projections in f32r/tf32 -- expected err ~1e-3):
  w1vT[u,v] = (values @ W1)^T   via PE f32r, bias folded later
  w2qT[u,q] = (query  @ W2)^T + (b1+b2)  (bias add on DVE, not ACT)
  per 16-q block:
    pre[u,(c,q,v)] = w1vT[u,(c,v)] + w2qT[u,(c,q)]   DVE tensor_scalar
      (bf16 src+dst SBUF -> 4x port mode)
    feat = tanh(pre)   ACT bf16 (the ~55us floor: 65536 el/part @ 1/cyc)
    score via PE bf16: per q-pair one [K=128,M=32,N=512] matmul per
      u-chunk with V placed in a 32-col window; tile_position=(0,32g)
      spreads pairs round-robin over the 4 PE column groups so 4 MMs run
      concurrently (pair p -> psum row 32(p%4)+p//4; both that row's
      cols 0:256 (even q) and 256:512 (odd q) hold its pair's scores).
  softmax: exp on ACT (|score|<=sum|V|~13, no max-sub); the row
  permutation is undone for free by transposing with 0/1 permutation
  matrices (PE matmul att_e.T @ Pe + att_o.T @ Po accumulated in psum);
  rowsums come from a ones-column matmul on attnT (natural q order), so
  no predicated merges are needed; context = attnT^T @ values (bf16),
  scaled by 1/rowsum on DVE.  bv is dropped: softmax is shift-invariant.
Engine budget per core: ACT ~58us (tanh+exp only), DVE ~50us (adds +
softmax glue), PE ~15us cold -- ACT is the roofline for this op.
"""
import sys
import numpy as np

if '/opt/trn_rl_repo' not in sys.path:
    sys.path.insert(0, '/opt/trn_rl_repo')

B, TQ, TV, D, U = 8, 128, 256, 512, 256
P = 128          # partitions
KD = D // P      # 4 k-chunks over d
CU = U // P      # 2 chunks over u
CV = TV // P     # 2 chunks over v

# q-blocks: (q0, bq, tanh split sizes).  Block 0 is split so the first
# tanh starts after only 4 q's of DVE adds; the last block is split so
# the final score matmuls (and exp) start with minimal ACT stall.
# (q0, bq, tanh split sizes, nfuse): the last `nfuse` q's of a block
# skip the DVE add -- ACT computes tanh(w1v + bias) fused.  Block 0
# fuses 6 q's: fused tiles need no DVE adds, so they fill the ACT
# pipe while DVE ramps up, hiding most of the projection latency.
BLOCKS = [(0, 16, [4, 4, 6], 2), (16, 16, [15], 1), (32, 16, [15], 1),
          (48, 16, [16], 0), (64, 16, [16], 0), (80, 16, [16], 0),
          (96, 16, [16], 0), (112, 16, [8, 4, 4], 0)]

_compiled = None


def _row_of_pair(p):
    """psum row for q-pair p (round-robin over the 4 PE col groups)."""
    return 32 * (p % 4) + (p // 4)


def _build():
    import concourse.bass as bass
    import concourse.tile as tile
    from concourse import bacc, mybir

    f32 = mybir.dt.float32
    f32r = mybir.dt.float32r
    bf16 = mybir.dt.bfloat16
    AF = mybir.ActivationFunctionType

    nc = bacc.Bacc("TRN2", target_bir_lowering=False, debug=False,
                   enable_asserts=True, num_devices=B)

    W12_d = nc.dram_tensor("W12", [P, KD, 2 * U], bf16,
                           kind="ExternalInput").ap()
    VQT_d = nc.dram_tensor("VQT", [P, KD, TV + TQ], bf16,
                           kind="ExternalInput").ap()
    VALS_d = nc.dram_tensor("VALS", [P, CV, D], bf16,
                            kind="ExternalInput").ap()
    VPAIR_d = nc.dram_tensor("VPAIR", [P, CU, 16, 32], bf16,
                             kind="ExternalInput").ap()
    PERM_d = nc.dram_tensor("PERM", [P, 2, TQ], bf16,
                            kind="ExternalInput").ap()
    ONES_d = nc.dram_tensor("ONES", [P, 1], bf16, kind="ExternalInput").ap()
    B12_d = nc.dram_tensor("B12", [P, CU], f32, kind="ExternalInput").ap()
    OUT_d = nc.dram_tensor("OUT", [TQ, D], f32, kind="ExternalOutput").ap()

    with tile.TileContext(nc) as tc:
        with (
            tc.tile_pool(name="cst", bufs=1) as cst,
            tc.tile_pool(name="pre_p", bufs=3) as pre_p,
            tc.tile_pool(name="feat_p", bufs=3) as feat_p,
            tc.tile_pool(name="sm", bufs=1) as sm,
            tc.tile_pool(name="ps", bufs=1, space=bass.MemorySpace.PSUM) as ps,
        ):
            # ---- inputs: one kick per tensor (a kick costs ~700ns of
            # engine time), balanced over the 3 DMA-capable queues; the
            # ACT table load (~2.7us, auto-inserted before the dummy
            # tanh) overlaps the w2 transfer ----
            dummy = cst.tile([P, 1], f32, tag="dummy")
            nc.gpsimd.memset(dummy[:], 0.0)
            w12 = cst.tile([P, KD, 2 * U], bf16, tag="w12")
            vqt = cst.tile([P, KD, TV + TQ], bf16, tag="vqt")
            w1 = w12[:, :, 0:U]
            w2 = w12[:, :, U:2 * U]
            vt = vqt[:, :, 0:TV]
            qt = vqt[:, :, TV:TV + TQ]
            b12 = cst.tile([P, CU], f32, tag="b12")
            vals = cst.tile([P, CV, D], bf16, tag="vals")
            vpair = cst.tile([P, CU, 16, 32], bf16, tag="vpair")
            perm = cst.tile([P, 2, TQ], bf16, tag="perm")
            ones = cst.tile([P, 1], bf16, tag="ones")
            # heavy tensors split by k-halves over 3 queues (each DMA
            # has ~3us fixed latency, so finer chunks don't help)
            nc.scalar.dma_start(w12[:, 2:4, :], W12_d[:, 2:4, :])
            nc.scalar.activation(dummy[:], dummy[:], AF.Tanh)
            nc.gpsimd.dma_start(w12[:, 0:2, :], W12_d[:, 0:2, :])
            nc.sync.dma_start(vqt[:, 0:2, :], VQT_d[:, 0:2, :])
            nc.sync.dma_start(vqt[:, 2:4, :], VQT_d[:, 2:4, :])
            nc.gpsimd.dma_start(b12[:], B12_d)
            nc.sync.dma_start(vals[:], VALS_d)
            nc.scalar.dma_start(vpair[:], VPAIR_d)
            nc.scalar.dma_start(perm[:], PERM_d)
            nc.scalar.dma_start(ones[:], ONES_d)

            # score psum banks hold garbage in never-written rows; the
            # perm matmul multiplies exp() of those rows by 0, so they
            # must be finite -> memset once.  attnT's unwritten half is
            # read (as don't-care) by the first ctx matmul -> memset.
            score_A = ps.tile([P, 2 * TV], f32, tag="scoreA")  # one bank
            score_B = ps.tile([P, 2 * TV], f32, tag="scoreB")  # one bank
            # zero the banks on the idle startup ACT (scale=0 copy);
            # src bytes are arbitrary (vqt lands first)
            nc.scalar.activation(score_A[:], vqt[:, 0:2, 0:TV], AF.Identity,
                                 scale=0.0)
            nc.scalar.activation(score_B[:], vqt[:, 0:2, 0:TV], AF.Identity,
                                 scale=0.0)
            attnT = sm.tile([P, CV, TQ], bf16, tag="attnT")
            nc.gpsimd.memset(attnT[:], 0.0)

            # ---- projections (PE bf16, 1 cyc/col, FWL weight loads) ----
            psW1 = ps.tile([P, CU, TV], f32, tag="psW1")       # one bank
            psW2 = ps.tile([P, CU, TQ], f32, tag="psW2")       # half bank
            w1vT = sm.tile([P, CU, TV], bf16, tag="w1vT")
            w2qT = sm.tile([P, CU, TQ], f32, tag="w2qT")

            # psW1 k-major so the first 4 matmuls only need k01 chunks
            i = 0
            for k in range(KD):
                for c in range(CU):
                    nc.tensor.matmul(psW1[:, c, :],
                                     w1[:, k, c * P:(c + 1) * P],
                                     vt[:, k, :],
                                     start=(i == 0), stop=(i == 2 * KD - 1))
                    i += 1
            for c in range(CU):
                nc.scalar.activation(w1vT[:, c, :], psW1[:, c, :], AF.Identity)
            # psW2 head (q 0:8) first so block 0's adds start early
            for lo, hi, g0 in ((0, 8, 0), (8, TQ, 1)):
                i = 0
                n = 2 * KD
                for k in range(KD):
                    for c in range(CU):
                        nc.tensor.matmul(psW2[:, c, lo:hi],
                                         w2[:, k, c * P:(c + 1) * P],
                                         qt[:, k, lo:hi],
                                         start=(i == 0), stop=(i == n - 1))
                        i += 1
                for c in range(CU):  # bias add on startup-idle ACT
                    nc.scalar.activation(w2qT[:, c, lo:hi], psW2[:, c, lo:hi],
                                         AF.Identity, bias=b12[:, c:c + 1])

            # ---- score / softmax / context ----
            # psT and the two rowsum columns share one bank (uses are
            # serialized: transpose-W -> attnT copy-R -> sums-W -> rcp-R)
            psTS = ps.tile([P, CV, TQ + 1], f32, tag="psT")    # one bank
            psT = psTS[:, :, 0:TQ]
            ctx_ps = ps.tile([P, D], f32, tag="ctx")           # one bank
            att_e = sm.tile([P, TV], bf16, tag="att_e")
            att_o = sm.tile([P, TV], bf16, tag="att_o")
            ctx = sm.tile([P, D], f32, tag="ctxsb")
            rcp = sm.tile([P, 2], f32, tag="rcp")

            def softmax_pe_act(half):
                """exp (ACT) + permuted transpose (PE).  For the final
                half, exp is split per v-chunk so each transpose starts
                as soon as its chunk's exp lands (shorter tail)."""
                h0 = half * 64
                score_ps = score_A if half == 0 else score_B
                for c in range(CV):
                    cs = slice(c * P, (c + 1) * P)
                    if half == 1 or c == 0:
                        nc.scalar.activation(att_e[:, cs], score_ps[:, cs],
                                             AF.Exp)
                        nc.scalar.activation(att_o[:, cs],
                                             score_ps[:, TV + c * P:
                                                      TV + (c + 1) * P],
                                             AF.Exp)
                    if half == 0 and c == 0:
                        # half A: one big exp pair (fewer ACT instrs)
                        nc.scalar.activation(att_e[:, P:TV],
                                             score_ps[:, P:TV], AF.Exp)
                        nc.scalar.activation(att_o[:, P:TV],
                                             score_ps[:, TV + P:2 * TV],
                                             AF.Exp)
                    nc.tensor.matmul(psT[:, c, h0:h0 + 64],
                                     att_e[:, cs],
                                     perm[:, 0, h0:h0 + 64],
                                     start=True, stop=False)
                    nc.tensor.matmul(psT[:, c, h0:h0 + 64],
                                     att_o[:, cs],
                                     perm[:, 1, h0:h0 + 64],
                                     start=False, stop=True)

            def softmax_dve(half):
                h0 = half * 64
                # per-chunk cast so sums/ctx start on c0 while c1 lands;
                # sums (tiny N=1 matmuls) go first so recip overlaps ctx
                for c in range(CV):
                    nc.vector.tensor_copy(attnT[:, c, h0:h0 + 64],
                                          psT[:, c, h0:h0 + 64])
                # ctx first (c0 only needs cast-c0); sums then recip
                # overlap the second ctx matmul.  full-M lhsT: the other
                # half's attnT columns are stale/zero -> don't-care rows
                for c in range(CV):
                    nc.tensor.matmul(ctx_ps[:], attnT[:, c, :],
                                     vals[:, c, :],
                                     start=(c == 0), stop=(c == CV - 1))
                for c in range(CV):
                    nc.tensor.matmul(psTS[:, half, TQ:TQ + 1], attnT[:, c, :],
                                     ones[:],
                                     start=(c == 0), stop=(c == CV - 1))
                nc.vector.reciprocal(rcp[h0:h0 + 64, half:half + 1],
                                     psTS[h0:h0 + 64, half, TQ:TQ + 1])
                if half == 0:
                    nc.vector.tensor_scalar_mul(ctx[h0:h0 + 64, :],
                                                ctx_ps[h0:h0 + 64, :],
                                                rcp[h0:h0 + 64, 0:1])
                    nc.sync.dma_start(OUT_d[h0:h0 + 64, :], ctx[h0:h0 + 64, :])
                else:
                    # tail: scale on the now-idle ACT, split output DMA
                    # across two queues
                    nc.scalar.activation(ctx[h0:h0 + 64, :],
                                         ctx_ps[h0:h0 + 64, :], AF.Identity,
                                         scale=rcp[h0:h0 + 64, 1:2])
                    nc.sync.dma_start(OUT_d[h0:h0 + 32, :],
                                      ctx[h0:h0 + 32, :])
                    nc.gpsimd.dma_start(OUT_d[h0 + 32:h0 + 64, :],
                                        ctx[h0 + 32:h0 + 64, :])

            mmA = [0, (TQ // 4) * CU]   # counter, total per score bank
            mmB = [0, (TQ // 4) * CU]
            for bi, (q0, bq, splits, nfuse) in enumerate(BLOCKS):
                assert sum(splits) == bq - nfuse
                # c-major layout: back-to-back dense TS adds measure
                # ~282ns here vs ~339 in q-major order (SBUF write/read
                # address interaction)
                pre = pre_p.tile([P, CU, 16, TV], bf16, tag="pre")
                feat = feat_p.tile([P, CU, 16, TV], bf16, tag="feat")
                # per-q tensor_scalar adds on DVE (2x_1p; TT-broadcast
                # and gpsimd variants measured slower); the last `nfuse`
                # q's skip DVE -- their add rides the ACT bias port
                for ql in range(bq - nfuse):
                    q = q0 + ql
                    for c in range(CU):
                        nc.vector.tensor_scalar_add(pre[:, c, ql, :],
                                                    w1vT[:, c, :],
                                                    w2qT[:, c, q:q + 1])
                # half-A softmax DVE tail goes after block 5's adds so
                # block 5's pre never stalls behind it in the DVE FIFO
                if bi == 5:
                    softmax_dve(0)
                s0 = 0
                for sl in splits:
                    nc.scalar.activation(feat[:, :, s0:s0 + sl, :],
                                         pre[:, :, s0:s0 + sl, :], AF.Tanh)
                    s0 += sl
                for ql in range(bq - nfuse, bq):   # fused add+tanh (ACT)
                    q = q0 + ql
                    for c in range(CU):
                        nc.scalar.activation(feat[:, c, ql, :],
                                             w1vT[:, c, :], AF.Tanh,
                                             bias=w2qT[:, c, q:q + 1])
                score_ps, mmc = (score_A, mmA) if q0 < 64 else (score_B, mmB)
                for pl in range(bq // 2):
                    pg = (q0 // 2) + pl
                    g, r = pg % 4, _row_of_pair(pg)
                    for c in range(CU):
                        nc.tensor.matmul(score_ps[32 * g:32 * g + 32, :],
                                         vpair[:, c, pg // 4, :],
                                         feat[:, c, 2 * pl:2 * pl + 2, :],
                                         start=(mmc[0] == 0),
                                         stop=(mmc[0] == mmc[1] - 1),
                                         tile_position=(0, 32 * g))
                        mmc[0] += 1
                # exp-A sits after tanh of block 4 in the ACT FIFO, so
                # block 3's score matmuls finish during that tanh and
                # exp-A starts stall-free
                if bi == 4:
                    softmax_pe_act(0)
            softmax_pe_act(1)
            softmax_dve(1)

    nc.compile()
    return nc


def _prep_shared(W1, b1, W2, b2, V, bv):
    import ml_dtypes
    bf16 = ml_dtypes.bfloat16
    Vf = np.asarray(V, np.float32)[:, 0]
    # V in a 32-col window per pair-slot j: col j <-> psum row 32g+j
    vpair = np.zeros((P, CU, 16, 32), np.float32)
    for c in range(CU):
        for j in range(16):
            vpair[:, c, j, j] = Vf[c * P:(c + 1) * P]
    # 0/1 permutation matrices: Pe[r, q]=1 (q even), Po[r, q]=1 (q odd)
    # where r = psum row of q's pair
    pe = np.zeros((P, 2, TQ), np.float32)
    for q in range(TQ):
        r = _row_of_pair(q // 2)
        pe[r, q % 2, q] = 1.0
    b12 = (np.asarray(b1) + np.asarray(b2)).astype(np.float32)
    b12 = b12.reshape(CU, P).T.copy()
    W1c = np.ascontiguousarray(
        np.asarray(W1, np.float32).reshape(KD, P, U).transpose(1, 0, 2))
    W2c = np.ascontiguousarray(
        np.asarray(W2, np.float32).reshape(KD, P, U).transpose(1, 0, 2))
    return {
        "W12": np.concatenate([W1c, W2c], axis=2).astype(bf16),
        "VPAIR": vpair.astype(bf16),
        "PERM": pe.astype(bf16),
        "ONES": np.ones((P, 1), bf16),
        "B12": np.ascontiguousarray(b12),
    }


def kernel(query, values, W1, b1, W2, b2, V, bv, _trace=False, _tmpdir=None):
    global _compiled
    import ml_dtypes
    from concourse.bass_utils import run_bass_kernel_spmd
    bf16 = ml_dtypes.bfloat16

    query = np.asarray(query, np.float32)
    values = np.asarray(values, np.float32)
    shared = _prep_shared(np.asarray(W1), np.asarray(b1), np.asarray(W2),
                          np.asarray(b2), np.asarray(V), np.asarray(bv))

    if _compiled is None:
        _compiled = _build()
    nc = _compiled

    in_maps = []
    for i in range(B):
        m = dict(shared)
        qT = query[i].T.reshape(KD, P, TQ).transpose(1, 0, 2)
        vT = values[i].T.reshape(KD, P, TV).transpose(1, 0, 2)
        vl = values[i].reshape(CV, P, D).transpose(1, 0, 2)
        m["VQT"] = np.concatenate([vT, qT], axis=2).astype(bf16)
        m["VALS"] = np.ascontiguousarray(vl).astype(bf16)
        in_maps.append(m)

    kw = {}
    if _trace:
        kw.update(trace=True, tmpdir=_tmpdir)
    res = run_bass_kernel_spmd(nc, in_maps, core_ids=list(range(B)), **kw)
    out = np.stack([res.results[i]["OUT"] for i in range(B)], axis=0)
    if _trace:
        kernel._last_trace = res
    return out
